# revision 1
# baseline (speedup 1.0000x reference)
"""MoE top-1 routing kernel for Trainium2, 8 NeuronCores.

Problem: x [2, 2048, 1024] f32; router w [1024, 4]; per-expert SwiGLU MLP
  gv = x @ w_v[e] ([1024, 8192]); h = silu(gv[:, :4096]) * gv[:, 4096:];
  y = h @ w_proj[e] ([4096, 1024]); out[t] = y_{argmax(router)}[t].

Sharding: tokens are dispatched by expert_idx at the host sharding step
(router is 0.03% of total FLOPs; argmax computed in f64, which matches the
f32 reference argmax exactly -- min top-2 logit gap for this data is ~3e-4,
far above f32 rounding noise). Tokens are permuted into expert-contiguous
order; every core receives ALL tokens (transposed, bf16) plus a 1/8 slice
of the hidden dimension of EVERY expert's weights (hidden-slice model
parallelism). Per-core work is therefore exactly total_tokens * (3*D*H/8)
MACs regardless of expert load imbalance, with zero capacity padding:
matmul free dims are the ragged per-expert block lengths.

Device program (identical SPMD; per-core weight inputs):
  for e in experts:  # token blocks of <=512 columns of the compact stream
    for hm in 4 gate/value 128-row tile pairs:
      psg = sum_k wv_gate[k] @ xT[k, blk]; psv = sum_k wv_val[k] @ xT[k, blk]
      ht[hm, blk] = silu(psg) * psv          (bf16)
    for blk: for d in 8: psy[d] = sum_k wp[d,k] @ ht[k, blk] -> yt (bf16)
Host combines: out = sum over cores of yt (f32), inverse-permuted.

All matmuls are bf16 (1 cycle/row at any free size on TRN2), PSUM f32.
PE work per core = 4096 tokens * 96 cycles = ~394k cycles = ~164 us.
"""

import sys

sys.path.insert(0, "/opt/trn_rl_repo")

import ml_dtypes
import numpy as np

import concourse.bass as bass  # noqa: F401  (kept for parity with utils)
import concourse.mybir as mybir
import concourse.tile as tile
from concourse import bacc
from concourse.bass_utils import run_bass_kernel_spmd

F32 = mybir.dt.float32
BF16 = mybir.dt.bfloat16
F8E4 = mybir.dt.float8e4
PM = mybir.MatmulPerfMode
AF = mybir.ActivationFunctionType
OP = mybir.AluOpType
BF16NP = np.dtype(ml_dtypes.bfloat16)
F8NP = np.dtype(ml_dtypes.float8_e4m3)
F8E5 = mybir.dt.float8e5
F8E5NP = np.dtype(ml_dtypes.float8_e5m2)

T = 4096      # tokens
D = 1024      # model dim
E = 4         # experts
H = 4096      # MLP hidden (SwiGLU: w_v outputs 2*H)
HS = H // 8   # hidden slice per core
KD = D // 128     # 8 k-tiles over model dim
MG = HS // 128    # 4 gate (and 4 value) 128-row tiles per slice
BLK = 512         # max token block (one PSUM bank of f32)
NWARM = 24        # PE warm-up dummy matmuls (cover the startup DMA window)
GP8 = 1           # leading 256-feature pairs of the GATE matmul in fp8
                  # DoubleRow (0 disables all fp8 paths)
HM8 = 2           # without CORR8: apply the fp8 gate pair for the last HM8
                  # hm tiles (each adds ~2.1e-2/sqrt(MG/HM8) rel error)
CORR8 = True      # x-residual correction: a 2nd DoubleRow (e5m2) computes
                  # w*x_lo, cancelling the x-quantization term of the main
                  # fp8 matmul. Residual error is w-quant only (~0.71x), so
                  # fp8 extends to ALL hm tiles at the same total error.
                  # (e0, hm0) stays bf16 to keep the startup path unchanged.
VHM8 = 3          # value-fp8 hm tiles per expert (e>0): per-element
                  # error = sqrt(4+VHM8)*0.665e-2 (+) 0.51e-2
S8 = 8.0          # fp8 pre-scale: weights*S8, x/S8 (keeps both operands in
                  # e4m3 normal range; products land at true scale)

# Expert loads for the seed-0 reference data (default build).
DEFAULT_COUNTS = (1149, 902, 974, 1071)


def _blocks(counts):
    """Static block structure: (expert, col_start, col_len) over the compact
    token stream; ragged tails, no padding. Expert 0 leads with a small
    128-col block so the PE can start ~3us earlier (first DMA is smaller)."""
    out = []
    c0 = 0
    for e in range(E):
        n = int(counts[e])
        off = 0
        if e == 0 and n > 256:
            out.append((e, c0, 256))
            off = 256
        while off < n:
            ln = min(BLK, n - off)
            out.append((e, c0 + off, ln))
            off += ln
        c0 += n
    return out


def _build(counts):
    nc = bacc.Bacc("TRN2", target_bir_lowering=False, debug=False, num_devices=8)

    xtr_d = nc.dram_tensor("xtr", [128, KD, T], BF16, kind="ExternalInput").ap()
    wvr_d = nc.dram_tensor(
        "wvr", [E * 2 * MG, 128, KD, 128], BF16, kind="ExternalInput"
    ).ap()
    wpr_d = nc.dram_tensor(
        "wpr", [E, 128, KD, MG, 128], BF16, kind="ExternalInput"
    ).ap()
    yt_d = nc.dram_tensor("yt", [128, KD, T], BF16, kind="ExternalOutput").ap()
    if GP8:
        xtr8_d = nc.dram_tensor(
            "xtr8", [128, GP8 * 2, T], F8E4, kind="ExternalInput"
        ).ap()
        wvg8_d = nc.dram_tensor(
            "wvg8", [E * MG, 128, GP8 * 2, 128], F8E4, kind="ExternalInput"
        ).ap()
        if CORR8:
            xtr8lo_d = nc.dram_tensor(
                "xtr8lo", [128, GP8 * 2, T], F8E5, kind="ExternalInput"
            ).ap()
            wvg8b_d = nc.dram_tensor(
                "wvg8b", [E * MG, 128, GP8 * 2, 128], F8E5,
                kind="ExternalInput",
            ).ap()
            wvv8_d = nc.dram_tensor(
                "wvv8", [E, 128, VHM8, GP8 * 2, 128], F8E4,
                kind="ExternalInput",
            ).ap()
            wvv8b_d = nc.dram_tensor(
                "wvv8b", [E, 128, VHM8, GP8 * 2, 128], F8E5,
                kind="ExternalInput",
            ).ap()

    blocks = _blocks(counts)

    with tile.TileContext(nc) as tc:
        with (
            tc.tile_pool(name="xte", bufs=1) as xp,
            tc.tile_pool(name="ht", bufs=1) as hp,
            tc.tile_pool(name="wv", bufs=6) as wvp,
            tc.tile_pool(name="wv8", bufs=4) as wv8p,
            tc.tile_pool(name="wp", bufs=2) as wpp,
            tc.tile_pool(name="act", bufs=3) as actp,
            tc.tile_pool(name="out", bufs=3) as outp,
            tc.tile_pool(name="pg", bufs=3, space="PSUM") as pg,
            tc.tile_pool(name="pv", bufs=2, space="PSUM") as pv,
            tc.tile_pool(name="py", bufs=3, space="PSUM") as py,
        ):
            xte = xp.tile([128, KD, T], BF16)
            ht = hp.tile([128, MG, T], BF16)
            if GP8:
                xte8 = xp.tile([128, GP8 * 2, T], F8E4)
                if CORR8:
                    xte8lo = xp.tile([128, GP8 * 2, T], F8E5)
            else:
                xte8 = None

            # PE warm-up: the Tensor engine runs at half clock until it has
            # been continuously busy for 3us. Dummy matmuls on a memset tile
            # keep it busy through the startup DMA window so all real
            # matmuls run at full p-state.
            warm = actp.tile([128, 128], BF16, tag="warm")
            nc.vector.memset(warm[:], 0.0)
            pwu = pg.tile([128, 128], F32, tag="g")
            for _ in range(NWARM):
                nc.tensor.matmul(
                    pwu[:], lhsT=warm[:], rhs=warm[:], start=True, stop=True
                )

            # Startup-critical DMAs on three different queues (per-DMA
            # sequencer time is ~0.6us, so serializing them on one queue
            # delays the first matmul); everything else in consumption order
            # on the SP queue.
            wv_tiles = {}
            wp_tiles = {}
            wv8_tiles = {}

            def load_wv(e, hm, eng_g=None, eng_l=None):
                wg = wvp.tile([128, KD, 128], BF16, tag="wg")
                (eng_g or nc.sync).dma_start(wg[:], wvr_d[e * 2 * MG + hm])
                wl = wvp.tile([128, KD, 128], BF16, tag="wl")
                (eng_l or nc.sync).dma_start(wl[:], wvr_d[e * 2 * MG + MG + hm])
                fp8_hm = (
                    GP8 and (not (e == 0 and hm == 0))
                    if CORR8
                    else GP8 and hm >= MG - HM8
                )
                wg8b = wl8 = wl8b = None
                if fp8_hm:
                    wg8 = wvp.tile([128, GP8 * 2, 128], F8E4, tag="wg8")
                    (eng_g or nc.sync).dma_start(wg8[:], wvg8_d[e * MG + hm])
                    if CORR8:
                        wg8b = wvp.tile(
                            [128, GP8 * 2, 128], F8E5, tag="wg8b"
                        )
                        (eng_g or nc.sync).dma_start(
                            wg8b[:], wvg8b_d[e * MG + hm]
                        )
                else:
                    wg8 = None
                wv_tiles[(e, hm)] = (wg, wl, wg8, wg8b)

            first_blk = blocks[0]
            # wl on the Pool SWDGE queue: its slower issue path lands it on
            # the shared DMA engines after the 2nd x chunk, which matches
            # consumption order (psv needs it ~0.9us after psg starts).
            load_wv(0, 0, eng_g=nc.sync, eng_l=nc.gpsimd)
            _, fc0, fln = first_blk
            nc.scalar.dma_start(
                xte[:, :, fc0 : fc0 + fln], xtr_d[:, :, fc0 : fc0 + fln]
            )
            for e in range(E):
                for (ee, c0, ln) in blocks:
                    if ee != e:
                        continue
                    if (ee, c0, ln) != first_blk:
                        nc.sync.dma_start(
                            xte[:, :, c0 : c0 + ln], xtr_d[:, :, c0 : c0 + ln]
                        )
                for hm in range(MG):
                    if (e, hm) in wv_tiles:
                        continue
                    load_wv(e, hm)
                # corrected fp8 VALUE weights for hm 2-3: one packed DMA per
                # tensor per expert (per-hm DMAs cost ~0.6us HWDGE each and
                # congest the queue); separate pool so the expert-lifetime
                # tile doesn't block the per-hm weight ring
                if CORR8:
                    # e0's weights ride the Pool/SWDGE queue (skips the
                    # shared HWDGE device) since its gv window is the one
                    # DMA-budget-bound stretch of the program
                    weng = nc.gpsimd if e == 0 else nc.sync
                    wl8p = wv8p.tile(
                        [128, VHM8, GP8 * 2, 128], F8E4, tag="wl8"
                    )
                    weng.dma_start(wl8p[:], wvv8_d[e])
                    wl8bp = wv8p.tile(
                        [128, VHM8, GP8 * 2, 128], F8E5, tag="wl8b"
                    )
                    weng.dma_start(wl8bp[:], wvv8b_d[e])
                    wv8_tiles[e] = (wl8p, wl8bp)
                # fp8 x copies: one merged DMA per tensor per expert (the
                # expert's columns are contiguous) — six tiny per-block DMAs
                # serialized ~3us of HWDGE overhead right where hm1's
                # DoubleRow needed the data. xte8lo on the Act queue to
                # halve the chain.
                if GP8:
                    ecols = [
                        (c0, ln) for (ee, c0, ln) in blocks if ee == e
                    ]
                    if ecols:
                        ec0 = ecols[0][0]
                        ec1 = ecols[-1][0] + ecols[-1][1]
                        nc.sync.dma_start(
                            xte8[:, :, ec0:ec1], xtr8_d[:, :, ec0:ec1]
                        )
                        if CORR8:
                            nc.scalar.dma_start(
                                xte8lo[:, :, ec0:ec1],
                                xtr8lo_d[:, :, ec0:ec1],
                            )
                wp_sb = wpp.tile([128, KD, MG, 128], BF16, tag="wp")
                nc.sync.dma_start(wp_sb[:], wpr_d[e])
                wp_tiles[e] = wp_sb

            for e in range(E):
                eblocks = [b for b in blocks if b[0] == e]
                # gate/value matmuls + silu-mult into ht
                for hm in range(MG):
                    wg, wl, wg8, wg8b = wv_tiles[(e, hm)]
                    fp8_here = wg8 is not None
                    # e0: only hm2-3 (its early window is DMA-bound)
                    fp8v_here = CORR8 and (
                        hm >= (MG - VHM8 if e > 0 else MG - 2)
                    )
                    if fp8v_here:
                        wl8p, wl8bp = wv8_tiles[e]
                        wl8 = wl8p[:, hm - (MG - VHM8), :, :]
                        wl8b = wl8bp[:, hm - (MG - VHM8), :, :]
                    for (_, c0, ln) in eblocks:
                        psg = pg.tile([128, BLK], F32, tag="g")
                        k0 = 2 * GP8 if fp8_here else 0
                        for k in range(k0, KD):
                            nc.tensor.matmul(
                                psg[:, :ln],
                                lhsT=wg[:, k, :],
                                rhs=xte[:, k, c0 : c0 + ln],
                                start=(k == k0),
                                stop=(k == KD - 1 and not fp8_here),
                                skip_group_check=True,
                            )
                        if fp8_here:
                            # leading feature pairs in fp8 DoubleRow (256-deep
                            # contraction at 0.5 cycles/output-row), last in
                            # the group so bf16 work never waits on fp8 inputs
                            nc.tensor.matmul(
                                psg[:, :ln],
                                lhsT=wg8[:, :, :],
                                rhs=xte8[:, :, c0 : c0 + ln],
                                start=False,
                                stop=not CORR8,
                                perf_mode=PM.DoubleRow,
                                skip_group_check=True,
                            )
                            if CORR8:
                                # e5m2 correction: accumulates w*x_lo, exactly
                                # cancelling the main DR's x-quantization term
                                nc.tensor.matmul(
                                    psg[:, :ln],
                                    lhsT=wg8b[:, :, :],
                                    rhs=xte8lo[:, :, c0 : c0 + ln],
                                    start=False,
                                    stop=True,
                                    perf_mode=PM.DoubleRow,
                                    skip_group_check=True,
                                )
                        psv = pv.tile([128, BLK], F32, tag="v")
                        kv0 = 2 * GP8 if fp8v_here else 0
                        for k in range(kv0, KD):
                            nc.tensor.matmul(
                                psv[:, :ln],
                                lhsT=wl[:, k, :],
                                rhs=xte[:, k, c0 : c0 + ln],
                                start=(k == kv0),
                                stop=(k == KD - 1 and not fp8v_here),
                                skip_group_check=True,
                            )
                        if fp8v_here:
                            nc.tensor.matmul(
                                psv[:, :ln],
                                lhsT=wl8,
                                rhs=xte8[:, :, c0 : c0 + ln],
                                start=False,
                                stop=False,
                                perf_mode=PM.DoubleRow,
                                skip_group_check=True,
                            )
                            nc.tensor.matmul(
                                psv[:, :ln],
                                lhsT=wl8b,
                                rhs=xte8lo[:, :, c0 : c0 + ln],
                                start=False,
                                stop=True,
                                perf_mode=PM.DoubleRow,
                                skip_group_check=True,
                            )
                        sact = actp.tile([128, BLK], F32, tag="s")
                        nc.scalar.activation(sact[:, :ln], psg[:, :ln], AF.Silu)
                        nc.vector.tensor_tensor(
                            out=ht[:, hm, c0 : c0 + ln],
                            in0=sact[:, :ln],
                            in1=psv[:, :ln],
                            op=OP.mult,
                        )
                # proj: per token block, all 8 d-tiles, one output DMA
                wp_sb = wp_tiles[e]
                for (_, c0, ln) in eblocks:
                    ysb = outp.tile([128, KD, BLK], BF16, tag="y")
                    is_last = (e, c0, ln) == blocks[-1]
                    for d in range(KD):
                        psy = py.tile([128, BLK], F32, tag="py")
                        for k in range(MG):
                            nc.tensor.matmul(
                                psy[:, :ln],
                                lhsT=wp_sb[:, d, k, :],
                                rhs=ht[:, k, c0 : c0 + ln],
                                start=(k == 0),
                                stop=(k == MG - 1),
                            )
                        if is_last and d % 2 == 1 and d != KD - 1:
                            # final block: alternate copies onto the idle Act
                            # engine so the exit chain isn't DVE-serialized
                            nc.scalar.activation(
                                ysb[:, d, :ln], psy[:, :ln], AF.Copy
                            )
                        else:
                            nc.vector.tensor_copy(ysb[:, d, :ln], psy[:, :ln])
                        if is_last and d == KD - 2:
                            # drain d0..6 early so only d7's copy + a tiny
                            # DMA sit on the critical tail
                            nc.scalar.dma_start(
                                yt_d[:, : KD - 1, c0 : c0 + ln],
                                ysb[:, : KD - 1, :ln],
                            )
                    if is_last:
                        nc.sync.dma_start(
                            yt_d[:, KD - 1 :, c0 : c0 + ln],
                            ysb[:, KD - 1 :, :ln],
                        )
                    else:
                        nc.scalar.dma_start(
                            yt_d[:, :, c0 : c0 + ln], ysb[:, :, :ln]
                        )

    nc.compile()
    return nc


_NC = None
_NC_COUNTS = None


def _route(x, w_router):
    """Host router: f64 logits argmax (exactly matches the f32 reference
    argmax for any non-degenerate top-2 gap)."""
    x2 = np.asarray(x, dtype=np.float64).reshape(T, D)
    logits = x2 @ np.asarray(w_router, dtype=np.float64)
    eidx = np.argmax(logits, axis=1)
    counts = np.bincount(eidx, minlength=E)
    order = np.argsort(eidx, kind="stable")
    return eidx, counts, order


def _get_nc(counts=DEFAULT_COUNTS):
    global _NC, _NC_COUNTS
    counts = tuple(int(c) for c in counts)
    if _NC is None or _NC_COUNTS != counts:
        _NC = _build(counts)
        _NC_COUNTS = counts
    return _NC


def make_in_maps(x, w_v, w_proj, order):
    x2 = np.asarray(x, dtype=np.float32).reshape(T, D)
    wv = np.asarray(w_v, dtype=np.float32)
    wp = np.asarray(w_proj, dtype=np.float32)

    # compact transposed x, bf16: xtr[p, k, t] = x[order[t], k*128+p]
    xT = np.ascontiguousarray(x2[order].T)  # [D, T]
    xtr = np.ascontiguousarray(
        xT.reshape(KD, 128, T).transpose(1, 0, 2).astype(BF16NP)
    )
    if GP8:
        # fp8 copy of the leading GP8*256 features, pre-scaled by 1/S8,
        # DoubleRow slot-major: xtr8[p, 2j+i, t] = x[t, j*256+i*128+p]/S8
        xh8 = (xT[: GP8 * 256] / S8).astype(F8NP)
        xtr8 = np.ascontiguousarray(
            xh8.reshape(GP8 * 2, 128, T).transpose(1, 0, 2)
        )
        if CORR8:
            # residual x_lo = x - dequant(x8): encoded e5m2 at the same
            # 1/S8 scale so (S8*w)*(x_lo/S8) accumulates at true scale
            xlo = xT[: GP8 * 256] - xh8.astype(np.float32) * S8
            xtr8lo = np.ascontiguousarray(
                (xlo / S8)
                .astype(F8E5NP)
                .reshape(GP8 * 2, 128, T)
                .transpose(1, 0, 2)
            )

    in_maps = []
    for c in range(8):
        h0 = c * HS
        wvr_e = []
        wpr_e = []
        wvg8_e = []
        wvv8_e = []
        for e in range(E):
            gate = wv[e][:, h0 : h0 + HS]                   # [D, HS]
            val = wv[e][:, H + h0 : H + h0 + HS]            # [D, HS]
            wv_my = np.concatenate([gate, val], axis=1)     # [D, 2*HS]
            # wvr[m, p, k, c2] = wv_my[k*128+p, m*128+c2]
            wvr_e.append(
                wv_my.reshape(KD, 128, 2 * MG, 128).transpose(2, 1, 0, 3)
            )
            wp_my = wp[e][h0 : h0 + HS, :]                  # [HS, D]
            # wpr[p, d, k, c2] = wp_my[k*128+p, d*128+c2]
            wpr_e.append(
                wp_my.reshape(MG, 128, KD, 128).transpose(1, 2, 0, 3)
            )
            if GP8:
                # wvg8[hm][p, 2j+i, m] = gate[j*256+i*128+p, hm*128+m]*S8
                g8 = (gate[: GP8 * 256] * S8).reshape(GP8 * 2, 128, MG, 128)
                wvg8_e.append(g8.transpose(2, 1, 0, 3))  # [MG, 128, 2G, 128]
                v8 = (val[: GP8 * 256] * S8).reshape(GP8 * 2, 128, MG, 128)
                # hm 2-3 only, packed [128, 2, 2G, 128] for one DMA/expert
                wvv8_e.append(
                    v8.transpose(2, 1, 0, 3)[MG - VHM8 :].transpose(1, 0, 2, 3)
                )
        wvr = np.ascontiguousarray(np.concatenate(wvr_e, axis=0).astype(BF16NP))
        wpr = np.ascontiguousarray(np.stack(wpr_e, axis=0).astype(BF16NP))
        im = {"xtr": xtr, "wvr": wvr, "wpr": wpr}
        if GP8:
            wvg8 = np.concatenate(wvg8_e, axis=0)
            im["xtr8"] = xtr8
            im["wvg8"] = np.ascontiguousarray(wvg8.astype(F8NP))
            if CORR8:
                wvv8 = np.stack(wvv8_e, axis=0)
                im["xtr8lo"] = xtr8lo
                im["wvg8b"] = np.ascontiguousarray(wvg8.astype(F8E5NP))
                im["wvv8"] = np.ascontiguousarray(wvv8.astype(F8NP))
                im["wvv8b"] = np.ascontiguousarray(wvv8.astype(F8E5NP))
        in_maps.append(im)
    return in_maps


def combine(results, order):
    """Sum the 8 hidden-slice partial outputs and inverse-permute."""
    ysum = np.zeros((128, KD, T), dtype=np.float32)
    for r in results:
        ysum += np.asarray(r["yt"]).astype(np.float32)
    yT = ysum.transpose(1, 0, 2).reshape(D, T)  # [D, T] compact order
    out = np.empty((T, D), dtype=np.float32)
    out[order] = yT.T
    return out.reshape(2, 2048, D)


def kernel(x, w_router, w_v, w_proj):
    eidx, counts, order = _route(x, w_router)
    nc = _get_nc(counts)
    in_maps = make_in_maps(x, w_v, w_proj, order)
    res = run_bass_kernel_spmd(nc, in_maps, core_ids=list(range(8)), trace=False)
    return combine(res.results, order)


if __name__ == "__main__":
    sys.path.insert(0, "/root/problem")
    import reference

    ins = {k: np.asarray(v) for k, v in reference.setup_inputs().items()}
    got = kernel(**ins)
    exp = np.asarray(reference.reference(**ins))
    err = np.abs(got - exp)
    denom = np.abs(exp).max()
    print("max abs err:", err.max(), "rel:", err.max() / denom)



# revision 3
# speedup vs baseline: 1.0785x; 1.0785x over previous
"""MoE top-1 routing kernel for Trainium2, 8 NeuronCores.

Problem: x [2, 2048, 1024] f32; router w [1024, 4]; per-expert SwiGLU MLP
  gv = x @ w_v[e] ([1024, 8192]); h = silu(gv[:, :4096]) * gv[:, 4096:];
  y = h @ w_proj[e] ([4096, 1024]); out[t] = y_{argmax(router)}[t].

Sharding: tokens are dispatched by expert_idx at the host sharding step
(router is 0.03% of total FLOPs; argmax computed in f64, which matches the
f32 reference argmax exactly). Tokens are permuted into expert-contiguous
order; every core receives ALL tokens plus a 1/8 slice of the hidden
dimension of EVERY expert's weights (hidden-slice model parallelism).
Per-core work is exactly total_tokens * (3*D*H/8) MACs regardless of
expert load imbalance, with zero capacity padding.

Numerics: every 256-deep contraction chunk of all three matmuls runs as
fp8 DoubleRow (0.5 PE cycles/row = 4x bf16) with a 3-term residual
decomposition that restores ~bf16 accuracy:
    main  e4m3(w*8)  x e4m3(x/8)     -- 4x-rate product of rounded values
    corrX e4m3(w*8)  x e5m2(x_lo/8)  -- cancels x's quantization error
    corrW e5m2(w_lo) x e4m3(x/8)     -- cancels w's quantization error
Residual error is (w_lo*x_lo + e5m2 rounding) ~ 2^-7 per element, measured
3.5e-3 max-rel end to end (threshold 2e-2). PE cost is 1.5 cycles per
256-chunk vs 2.0 for bf16: 72 cycles/token/core -> ~123us @2.4GHz.

The proj input h = silu(g)*v is produced on-device as the same e4m3+e5m2
pair: Act silu (psg->f32), DVE mult (x psv -> f32), Act copy->e4m3,
DVE subtract->e5m2. Proj PSUM->SBUF copies run on the otherwise idle Pool
engine. Output yt stores 8*y (w_proj is pre-scaled by 8); the host
combine divides by 8 after summing the 8 hidden-slice partials.
"""

import sys

sys.path.insert(0, "/opt/trn_rl_repo")

import ml_dtypes
import numpy as np

import concourse.bass as bass  # noqa: F401  (kept for parity with utils)
import concourse.mybir as mybir
import concourse.tile as tile
from concourse import bacc
from concourse.bass_utils import run_bass_kernel_spmd

F32 = mybir.dt.float32
BF16 = mybir.dt.bfloat16
F8E4 = mybir.dt.float8e4
F8E5 = mybir.dt.float8e5
PM = mybir.MatmulPerfMode
AF = mybir.ActivationFunctionType
OP = mybir.AluOpType
BF16NP = np.dtype(ml_dtypes.bfloat16)
F8NP = np.dtype(ml_dtypes.float8_e4m3)
F8E5NP = np.dtype(ml_dtypes.float8_e5m2)

T = 4096      # tokens
D = 1024      # model dim
E = 4         # experts
H = 4096      # MLP hidden (SwiGLU: w_v outputs 2*H)
HS = H // 8   # hidden slice per core
NCH = D // 256    # 4 fp8 256-chunks over model dim
PCH = HS // 256   # 2 fp8 256-chunks over the hidden slice (proj contraction)
MG = HS // 128    # 4 gate (and 4 value) 128-row tiles per slice
KD = D // 128     # 8 output d-tiles for proj
BLK = 512         # max token block (one PSUM bank of f32)
NWARM = 24        # PE warm-up dummy matmuls (cover the startup DMA window)
S8 = 8.0          # fp8 pre-scale: weights*S8, x/S8

# Expert loads for the seed-0 reference data (default build).
DEFAULT_COUNTS = (1149, 902, 974, 1071)


def _blocks(counts):
    """Static block structure: (expert, col_start, col_len) over the compact
    token stream; ragged tails, no padding. Expert 0 leads with a small
    256-col block so the PE can start earlier (first DMA is smaller)."""
    out = []
    c0 = 0
    for e in range(E):
        n = int(counts[e])
        off = 0
        if e == 0 and n > 256:
            out.append((e, c0, 256))
            off = 256
        while off < n:
            ln = min(BLK, n - off)
            out.append((e, c0 + off, ln))
            off += ln
        c0 += n
    return out


def _build(counts):
    nc = bacc.Bacc("TRN2", target_bir_lowering=False, debug=False, num_devices=8)

    xtr8_d = nc.dram_tensor("xtr8", [128, NCH, 2, T], F8E4, kind="ExternalInput").ap()
    xtr8lo_d = nc.dram_tensor(
        "xtr8lo", [128, NCH, 2, T], F8E5, kind="ExternalInput"
    ).ap()
    wv8_d = nc.dram_tensor(
        "wv8", [E * MG, 128, 2, NCH, 2, 128], F8E4, kind="ExternalInput"
    ).ap()
    wv8lo_d = nc.dram_tensor(
        "wv8lo", [E * MG, 128, 2, NCH, 2, 128], F8E5, kind="ExternalInput"
    ).ap()
    wp8_d = nc.dram_tensor(
        "wp8", [E, 128, KD, PCH, 2, 128], F8E4, kind="ExternalInput"
    ).ap()
    wp8lo_d = nc.dram_tensor(
        "wp8lo", [E, 128, KD, PCH, 2, 128], F8E5, kind="ExternalInput"
    ).ap()
    yt_d = nc.dram_tensor("yt", [128, KD, T], BF16, kind="ExternalOutput").ap()

    blocks = _blocks(counts)

    with tile.TileContext(nc) as tc:
        with (
            tc.tile_pool(name="xte", bufs=1) as xp,
            tc.tile_pool(name="ht", bufs=1) as hp,
            tc.tile_pool(name="wv", bufs=4) as wvp,
            tc.tile_pool(name="wp", bufs=2) as wpp,
            tc.tile_pool(name="act", bufs=3) as actp,
            tc.tile_pool(name="out", bufs=3) as outp,
            tc.tile_pool(name="pg", bufs=3, space="PSUM") as pg,
            tc.tile_pool(name="pv", bufs=2, space="PSUM") as pv,
            tc.tile_pool(name="py", bufs=3, space="PSUM") as py,
        ):
            xte8 = xp.tile([128, NCH, 2, T], F8E4)
            xte8lo = xp.tile([128, NCH, 2, T], F8E5)
            ht8 = hp.tile([128, MG, T], F8E4)
            ht8lo = hp.tile([128, MG, T], F8E5)

            # PE warm-up: the Tensor engine runs at half clock until it has
            # been continuously busy for 3us. Dummy matmuls on a memset tile
            # keep it busy through the startup DMA window so all real
            # matmuls run at full p-state.
            warm = actp.tile([128, 128], BF16, tag="warm")
            nc.vector.memset(warm[:], 0.0)
            pwu = pg.tile([128, 128], F32, tag="g")
            for _ in range(NWARM):
                nc.tensor.matmul(
                    pwu[:], lhsT=warm[:], rhs=warm[:], start=True, stop=True
                )

            # Startup-critical DMAs on different queues (per-DMA sequencer
            # time is ~0.6us, so serializing them on one queue delays the
            # first matmul); everything else in consumption order.
            wv_tiles = {}
            wp_tiles = {}

            def load_wv(e, hm, eng8=None, englo=None):
                w8 = wvp.tile([128, 2, NCH, 2, 128], F8E4, tag="w8")
                (eng8 or nc.sync).dma_start(w8[:], wv8_d[e * MG + hm])
                w8lo = wvp.tile([128, 2, NCH, 2, 128], F8E5, tag="w8lo")
                (englo or nc.sync).dma_start(w8lo[:], wv8lo_d[e * MG + hm])
                wv_tiles[(e, hm)] = (w8, w8lo)

            first_blk = blocks[0]
            # wv8lo(0,0) rides the Pool SWDGE queue (skips the shared HWDGE
            # devices); it is consumed ~0.5us after the first group starts.
            load_wv(0, 0, eng8=nc.sync, englo=nc.gpsimd)
            _, fc0, fln = first_blk
            nc.scalar.dma_start(
                xte8[:, :, :, fc0 : fc0 + fln], xtr8_d[:, :, :, fc0 : fc0 + fln]
            )
            nc.scalar.dma_start(
                xte8lo[:, :, :, fc0 : fc0 + fln],
                xtr8lo_d[:, :, :, fc0 : fc0 + fln],
            )
            for e in range(E):
                for (ee, c0, ln) in blocks:
                    if ee != e or (ee, c0, ln) == first_blk:
                        continue
                    nc.sync.dma_start(
                        xte8[:, :, :, c0 : c0 + ln], xtr8_d[:, :, :, c0 : c0 + ln]
                    )
                for hm in range(MG):
                    if (e, hm) in wv_tiles:
                        continue
                    load_wv(e, hm)
                # x-residual for the expert's columns: one merged DMA
                ecols = [(c0, ln) for (ee, c0, ln) in blocks if ee == e]
                ec0 = ecols[0][0]
                ec1 = ecols[-1][0] + ecols[-1][1]
                if e == 0:
                    ec0 = fc0 + fln  # block 0 already in flight on scalar
                if ec1 > ec0:
                    nc.scalar.dma_start(
                        xte8lo[:, :, :, ec0:ec1], xtr8lo_d[:, :, :, ec0:ec1]
                    )
                wp8_sb = wpp.tile([128, KD, PCH, 2, 128], F8E4, tag="wp8")
                nc.scalar.dma_start(wp8_sb[:], wp8_d[e])
                wp8lo_sb = wpp.tile([128, KD, PCH, 2, 128], F8E5, tag="wp8lo")
                nc.scalar.dma_start(wp8lo_sb[:], wp8lo_d[e])
                wp_tiles[e] = (wp8_sb, wp8lo_sb)

            for e in range(E):
                eblocks = [b for b in blocks if b[0] == e]
                # gate/value matmuls + silu-mult into ht8/ht8lo
                for hm in range(MG):
                    w8, w8lo = wv_tiles[(e, hm)]
                    for (_, c0, ln) in eblocks:
                        psg = pg.tile([128, BLK], F32, tag="g")
                        psv = pv.tile([128, BLK], F32, tag="v")
                        for gv, ps in ((0, psg), (1, psv)):
                            for c in range(NCH):
                                nc.tensor.matmul(
                                    ps[:, :ln],
                                    lhsT=w8[:, gv, c, :, :],
                                    rhs=xte8[:, c, :, c0 : c0 + ln],
                                    start=(c == 0),
                                    stop=False,
                                    perf_mode=PM.DoubleRow,
                                    skip_group_check=True,
                                )
                            for c in range(NCH):
                                # cancels the x-quantization error of main
                                nc.tensor.matmul(
                                    ps[:, :ln],
                                    lhsT=w8[:, gv, c, :, :],
                                    rhs=xte8lo[:, c, :, c0 : c0 + ln],
                                    start=False,
                                    stop=False,
                                    perf_mode=PM.DoubleRow,
                                    skip_group_check=True,
                                )
                            for c in range(NCH):
                                # cancels the w-quantization error of main
                                nc.tensor.matmul(
                                    ps[:, :ln],
                                    lhsT=w8lo[:, gv, c, :, :],
                                    rhs=xte8[:, c, :, c0 : c0 + ln],
                                    start=False,
                                    stop=(c == NCH - 1),
                                    perf_mode=PM.DoubleRow,
                                    skip_group_check=True,
                                )
                        sact = actp.tile([128, BLK], F32, tag="s")
                        nc.scalar.activation(sact[:, :ln], psg[:, :ln], AF.Silu)
                        h32 = actp.tile([128, BLK], F32, tag="h")
                        nc.vector.tensor_tensor(
                            out=h32[:, :ln],
                            in0=sact[:, :ln],
                            in1=psv[:, :ln],
                            op=OP.mult,
                        )
                        nc.scalar.activation(
                            ht8[:, hm, c0 : c0 + ln], h32[:, :ln], AF.Copy
                        )
                        nc.vector.tensor_tensor(
                            out=ht8lo[:, hm, c0 : c0 + ln],
                            in0=h32[:, :ln],
                            in1=ht8[:, hm, c0 : c0 + ln],
                            op=OP.subtract,
                        )
                # proj: per token block, all 8 d-tiles; copies on Pool
                wp8_sb, wp8lo_sb = wp_tiles[e]
                for (_, c0, ln) in eblocks:
                    ysb = outp.tile([128, KD, BLK], BF16, tag="y")
                    is_last = (e, c0, ln) == blocks[-1]
                    for d in range(KD):
                        psy = py.tile([128, BLK], F32, tag="py")
                        for c in range(PCH):
                            nc.tensor.matmul(
                                psy[:, :ln],
                                lhsT=wp8_sb[:, d, c, :, :],
                                rhs=ht8[:, 2 * c : 2 * c + 2, c0 : c0 + ln],
                                start=(c == 0),
                                stop=False,
                                perf_mode=PM.DoubleRow,
                                skip_group_check=True,
                            )
                        for c in range(PCH):
                            nc.tensor.matmul(
                                psy[:, :ln],
                                lhsT=wp8_sb[:, d, c, :, :],
                                rhs=ht8lo[:, 2 * c : 2 * c + 2, c0 : c0 + ln],
                                start=False,
                                stop=False,
                                perf_mode=PM.DoubleRow,
                                skip_group_check=True,
                            )
                        for c in range(PCH):
                            nc.tensor.matmul(
                                psy[:, :ln],
                                lhsT=wp8lo_sb[:, d, c, :, :],
                                rhs=ht8[:, 2 * c : 2 * c + 2, c0 : c0 + ln],
                                start=False,
                                stop=(c == PCH - 1),
                                perf_mode=PM.DoubleRow,
                                skip_group_check=True,
                            )
                        if is_last and d % 2 == 1 and d != KD - 1:
                            # final block: alternate copies onto Act so the
                            # exit chain isn't serialized on one engine
                            nc.scalar.activation(
                                ysb[:, d, :ln], psy[:, :ln], AF.Copy
                            )
                        else:
                            nc.gpsimd.tensor_copy(ysb[:, d, :ln], psy[:, :ln])
                        if is_last and d == KD - 2:
                            # drain d0..6 early so only d7's copy + a tiny
                            # DMA sit on the critical tail
                            nc.scalar.dma_start(
                                yt_d[:, : KD - 1, c0 : c0 + ln],
                                ysb[:, : KD - 1, :ln],
                            )
                    if is_last:
                        nc.sync.dma_start(
                            yt_d[:, KD - 1 :, c0 : c0 + ln],
                            ysb[:, KD - 1 :, :ln],
                        )
                    else:
                        nc.scalar.dma_start(
                            yt_d[:, :, c0 : c0 + ln], ysb[:, :, :ln]
                        )

    nc.compile()
    return nc


_NC = None
_NC_COUNTS = None


def _route(x, w_router):
    """Host router: f64 logits argmax (exactly matches the f32 reference
    argmax for any non-degenerate top-2 gap)."""
    x2 = np.asarray(x, dtype=np.float64).reshape(T, D)
    logits = x2 @ np.asarray(w_router, dtype=np.float64)
    eidx = np.argmax(logits, axis=1)
    counts = np.bincount(eidx, minlength=E)
    order = np.argsort(eidx, kind="stable")
    return eidx, counts, order


def _get_nc(counts=DEFAULT_COUNTS):
    global _NC, _NC_COUNTS
    counts = tuple(int(c) for c in counts)
    if _NC is None or _NC_COUNTS != counts:
        _NC = _build(counts)
        _NC_COUNTS = counts
    return _NC


def _split8(a):
    """hi = e4m3(a), lo = e5m2(a - hi); both at the caller's pre-scale."""
    hi = a.astype(F8NP)
    lo = (a - hi.astype(np.float32)).astype(F8E5NP)
    return hi, lo


def make_in_maps(x, w_v, w_proj, order):
    x2 = np.asarray(x, dtype=np.float32).reshape(T, D)
    wv = np.asarray(w_v, dtype=np.float32)
    wp = np.asarray(w_proj, dtype=np.float32)

    # compact transposed x: xtr8[p, c, j, t] = x[order[t], c*256+j*128+p] / 8
    xT = np.ascontiguousarray(x2[order].T)  # [D, T]
    x8, x8lo = _split8(xT / np.float32(S8))
    xtr8 = np.ascontiguousarray(
        x8.reshape(NCH, 2, 128, T).transpose(2, 0, 1, 3)
    )
    xtr8lo = np.ascontiguousarray(
        x8lo.reshape(NCH, 2, 128, T).transpose(2, 0, 1, 3)
    )

    in_maps = []
    for core in range(8):
        h0 = core * HS
        wv8_e = []
        wv8lo_e = []
        wp8_e = []
        wp8lo_e = []
        for e in range(E):
            for hm in range(MG):
                lo = h0 + hm * 128
                gcols = wv[e][:, lo : lo + 128]
                vcols = wv[e][:, H + lo : H + lo + 128]
                st = np.stack([gcols, vcols])  # [2, D, 128]
                q8, q8lo = _split8(st * np.float32(S8))
                # [gv, c, j, p, m] -> [p, gv, c, j, m]
                wv8_e.append(
                    q8.reshape(2, NCH, 2, 128, 128).transpose(3, 0, 1, 2, 4)
                )
                wv8lo_e.append(
                    q8lo.reshape(2, NCH, 2, 128, 128).transpose(3, 0, 1, 2, 4)
                )
            wp_my = wp[e][h0 : h0 + HS, :]  # [HS, D]
            p8, p8lo = _split8(wp_my * np.float32(S8))
            # [c, j, p, d, m] -> [p, d, c, j, m]
            wp8_e.append(
                p8.reshape(PCH, 2, 128, KD, 128).transpose(2, 3, 0, 1, 4)
            )
            wp8lo_e.append(
                p8lo.reshape(PCH, 2, 128, KD, 128).transpose(2, 3, 0, 1, 4)
            )
        im = {
            "xtr8": xtr8,
            "xtr8lo": xtr8lo,
            "wv8": np.ascontiguousarray(np.stack(wv8_e)),
            "wv8lo": np.ascontiguousarray(np.stack(wv8lo_e)),
            "wp8": np.ascontiguousarray(np.stack(wp8_e)),
            "wp8lo": np.ascontiguousarray(np.stack(wp8lo_e)),
        }
        in_maps.append(im)
    return in_maps


def combine(results, order):
    """Sum the 8 hidden-slice partial outputs (each 8*y), inverse-permute,
    and divide out the w_proj pre-scale."""
    ysum = np.zeros((128, KD, T), dtype=np.float32)
    for r in results:
        ysum += np.asarray(r["yt"]).astype(np.float32)
    ysum *= np.float32(1.0 / S8)
    yT = ysum.transpose(1, 0, 2).reshape(D, T)  # [D, T] compact order
    out = np.empty((T, D), dtype=np.float32)
    out[order] = yT.T
    return out.reshape(2, 2048, D)


def kernel(x, w_router, w_v, w_proj):
    eidx, counts, order = _route(x, w_router)
    nc = _get_nc(counts)
    in_maps = make_in_maps(x, w_v, w_proj, order)
    res = run_bass_kernel_spmd(nc, in_maps, core_ids=list(range(8)), trace=False)
    return combine(res.results, order)


if __name__ == "__main__":
    sys.path.insert(0, "/root/problem")
    import reference

    ins = {k: np.asarray(v) for k, v in reference.setup_inputs().items()}
    got = kernel(**ins)
    exp = np.asarray(reference.reference(**ins))
    err = np.abs(got - exp)
    denom = np.abs(exp).max()
    print("max abs err:", err.max(), "rel:", err.max() / denom)


# revision 9
# speedup vs baseline: 1.2669x; 1.1747x over previous
"""MoE top-1 routing kernel for Trainium2, 8 NeuronCores.

Problem: x [2, 2048, 1024] f32; router w [1024, 4]; per-expert SwiGLU MLP
  gv = x @ w_v[e] ([1024, 8192]); h = silu(gv[:, :4096]) * gv[:, 4096:];
  y = h @ w_proj[e] ([4096, 1024]); out[t] = y_{argmax(router)}[t].

Sharding: tokens are dispatched by expert_idx at the host sharding step
(router is 0.03% of total FLOPs; argmax computed in f64, which matches the
f32 reference argmax exactly). Tokens are permuted into expert-contiguous
order; every core receives ALL tokens plus a 1/8 slice of the hidden
dimension of EVERY expert's weights (hidden-slice model parallelism).
Per-core work is exactly total_tokens * (3*D*H/8) MACs regardless of
expert load imbalance, with zero capacity padding.

Numerics: every 256-deep contraction chunk of all three matmuls runs as
fp8 DoubleRow (0.5 PE cycles/row = 4x bf16) with a 3-term residual
decomposition that restores ~bf16 accuracy:
    main  e4m3(w*8)  x e4m3(x/8)     -- 4x-rate product of rounded values
    corrX e4m3(w*8)  x e5m2(x_lo/8)  -- cancels x's quantization error
    corrW e5m2(w_lo) x e4m3(x/8)     -- cancels w's quantization error
Residual error is (w_lo*x_lo + e5m2 rounding) ~ 2^-7 per element, measured
3.5e-3 max-rel end to end (threshold 2e-2). PE cost is 1.5 cycles per
256-chunk vs 2.0 for bf16: 72 cycles/token/core -> ~123us @2.4GHz.

The proj input h = silu(g)*v is produced on-device as the same e4m3+e5m2
pair: Act silu (psg->f32), DVE mult (x psv -> f32), Act copy->e4m3,
DVE subtract->e5m2. Proj PSUM->SBUF copies run on the otherwise idle Pool
engine. Output yt stores 8*y (w_proj is pre-scaled by 8); the host
combine divides by 8 after summing the 8 hidden-slice partials.
"""

import sys

sys.path.insert(0, "/opt/trn_rl_repo")

import ml_dtypes
import numpy as np

import concourse.bass as bass  # noqa: F401  (kept for parity with utils)
import concourse.mybir as mybir
import concourse.tile as tile
from concourse import bacc
from concourse.bass_utils import run_bass_kernel_spmd

F32 = mybir.dt.float32
BF16 = mybir.dt.bfloat16
F8E4 = mybir.dt.float8e4
F8E5 = mybir.dt.float8e5
PM = mybir.MatmulPerfMode
AF = mybir.ActivationFunctionType
OP = mybir.AluOpType
BF16NP = np.dtype(ml_dtypes.bfloat16)
F8NP = np.dtype(ml_dtypes.float8_e4m3)
F8E5NP = np.dtype(ml_dtypes.float8_e5m2)

T = 4096      # tokens
D = 1024      # model dim
E = 4         # experts
H = 4096      # MLP hidden (SwiGLU: w_v outputs 2*H)
HS = H // 8   # hidden slice per core
NCH = D // 256    # 4 fp8 256-chunks over model dim
PCH = HS // 256   # 2 fp8 256-chunks over the hidden slice (proj contraction)
MG = HS // 128    # 4 gate (and 4 value) 128-row tiles per slice
KD = D // 128     # 8 output d-tiles for proj
BLK = 512         # max token block (one PSUM bank of f32)
NWARM = 24        # PE warm-up dummy matmuls (cover the startup DMA window)
S8 = 8.0          # fp8 pre-scale: weights*S8, x/S8

# Per-token error units where the w-residual correction is skipped (the
# chunk runs 2-term: main + x-residual only). Each unit saves 0.5 PE
# cycles/token (~0.85us) and adds ~0.64e-2 (RSS) to the end-to-end rel
# error on top of the ~0.35e-2 3-term base. 6 units -> ~1.6e-2 predicted.
SKIP_W_GATE = {(0, 0), (1, 0), (2, 0), (3, 0)}   # (hm, chunk)
SKIP_W_VAL = {(0, 0), (1, 0)}

# Expert loads for the seed-0 reference data (default build).
DEFAULT_COUNTS = (1149, 902, 974, 1071)


def _blocks(counts):
    """Static block structure: (expert, col_start, col_len) over the compact
    token stream; ragged tails, no padding. Expert 0 leads with a small
    256-col block so the PE can start earlier (first DMA is smaller)."""
    out = []
    c0 = 0
    for e in range(E):
        n = int(counts[e])
        off = 0
        if e == 0 and n > 256:
            out.append((e, c0, 256))
            off = 256
        while off < n:
            ln = min(BLK, n - off)
            out.append((e, c0 + off, ln))
            off += ln
        c0 += n
    return out


def _build(counts):
    nc = bacc.Bacc("TRN2", target_bir_lowering=False, debug=False, num_devices=8)

    xtr8_d = nc.dram_tensor("xtr8", [128, NCH, 2, T], F8E4, kind="ExternalInput").ap()
    xtr8lo_d = nc.dram_tensor(
        "xtr8lo", [128, NCH, 2, T], F8E5, kind="ExternalInput"
    ).ap()
    wv8_d = nc.dram_tensor(
        "wv8", [E * MG, 128, 2, NCH, 2, 128], F8E4, kind="ExternalInput"
    ).ap()
    wv8lo_d = nc.dram_tensor(
        "wv8lo", [E * MG, 128, 2, NCH, 2, 128], F8E5, kind="ExternalInput"
    ).ap()
    wp8_d = nc.dram_tensor(
        "wp8", [E, 128, KD, PCH, 2, 128], F8E4, kind="ExternalInput"
    ).ap()
    wp8lo_d = nc.dram_tensor(
        "wp8lo", [E, 128, KD, PCH, 2, 128], F8E5, kind="ExternalInput"
    ).ap()
    yt_d = nc.dram_tensor("yt", [128, KD, T], BF16, kind="ExternalOutput").ap()

    blocks = _blocks(counts)

    with tile.TileContext(nc) as tc:
        with (
            tc.tile_pool(name="xte", bufs=1) as xp,
            tc.tile_pool(name="ht", bufs=1) as hp,
            tc.tile_pool(name="wv", bufs=4) as wvp,
            tc.tile_pool(name="wp", bufs=2) as wpp,
            tc.tile_pool(name="act", bufs=3) as actp,
            tc.tile_pool(name="out", bufs=3) as outp,
            tc.tile_pool(name="pg", bufs=3, space="PSUM") as pg,
            tc.tile_pool(name="pv", bufs=2, space="PSUM") as pv,
            tc.tile_pool(name="py", bufs=3, space="PSUM") as py,
        ):
            xte8 = xp.tile([128, NCH, 2, T], F8E4)
            xte8lo = xp.tile([128, NCH, 2, T], F8E5)
            ht8 = hp.tile([128, MG, T], F8E4)
            ht8lo = hp.tile([128, MG, T], F8E5)

            # PE warm-up: the Tensor engine runs at half clock until it has
            # been continuously busy for 3us. Dummy matmuls on a memset tile
            # keep it busy through the startup DMA window so all real
            # matmuls run at full p-state.
            warm = actp.tile([128, 128], BF16, tag="warm")
            nc.vector.memset(warm[:], 0.0)
            pwu = pg.tile([128, 128], F32, tag="g")
            for _ in range(NWARM):
                nc.tensor.matmul(
                    pwu[:], lhsT=warm[:], rhs=warm[:], start=True, stop=True
                )

            # Startup-critical DMAs on different queues (per-DMA sequencer
            # time is ~0.6us, so serializing them on one queue delays the
            # first matmul); everything else in consumption order.
            wv_tiles = {}
            wp_tiles = {}

            def load_wv(e, hm, eng8=None, englo=None):
                w8 = wvp.tile([128, 2, NCH, 2, 128], F8E4, tag="w8")
                (eng8 or nc.sync).dma_start(w8[:], wv8_d[e * MG + hm])
                w8lo = wvp.tile([128, 2, NCH, 2, 128], F8E5, tag="w8lo")
                (englo or nc.sync).dma_start(w8lo[:], wv8lo_d[e * MG + hm])
                wv_tiles[(e, hm)] = (w8, w8lo)

            first_blk = blocks[0]
            _, fc0, fln = first_blk
            e0b = [(c0, ln) for (ee, c0, ln) in blocks if ee == 0]
            # Startup: e0 runs hm0/hm1 interleaved per block, so the first
            # weights needed are (0,0) and (0,1). Their e4m3 parts lead the
            # SP queue. The Pool SWDGE queue (idle until e0's proj) leads
            # with block 0's x-residual -- the first correction input the
            # PE needs -- then the two weight residuals and block 1's
            # x-residual, all consumed a few DRs later. x block 0 rides the
            # Act queue behind the Silu table load.
            nc.gpsimd.dma_start(
                xte8lo[:, :, :, fc0 : fc0 + fln],
                xtr8lo_d[:, :, :, fc0 : fc0 + fln],
            )
            load_wv(0, 0, eng8=nc.sync, englo=nc.gpsimd)
            load_wv(0, 1, eng8=nc.sync, englo=nc.gpsimd)
            nc.scalar.dma_start(
                xte8[:, :, :, fc0 : fc0 + fln], xtr8_d[:, :, :, fc0 : fc0 + fln]
            )
            if len(e0b) > 1:
                b1c0, b1ln = e0b[1]
                nc.gpsimd.dma_start(
                    xte8lo[:, :, :, b1c0 : b1c0 + b1ln],
                    xtr8lo_d[:, :, :, b1c0 : b1c0 + b1ln],
                )
            # Remaining x-residuals on the Pool SWDGE queue: Pool's compute
            # (proj PSUM->SBUF copies) doesn't start until the first proj
            # block (~26us), so its queue is free during the load window.
            xlo_spans = []
            for e in range(E):
                ecols = [(c0, ln) for (ee, c0, ln) in blocks if ee == e]
                ec0 = ecols[0][0]
                ec1 = ecols[-1][0] + ecols[-1][1]
                if e == 0:
                    # blocks 0 and 1 already in flight on the Act queue
                    ec0 = min(ec1, e0b[1][0] + e0b[1][1] if len(e0b) > 1 else ec1)
                if ec1 > ec0:
                    xlo_spans.append((ec0, ec1))
            for (a, b) in xlo_spans:
                nc.gpsimd.dma_start(
                    xte8lo[:, :, :, a:b], xtr8lo_d[:, :, :, a:b]
                )

            for e in range(E):
                for (ee, c0, ln) in blocks:
                    if ee != e or (ee, c0, ln) == first_blk:
                        continue
                    nc.sync.dma_start(
                        xte8[:, :, :, c0 : c0 + ln], xtr8_d[:, :, :, c0 : c0 + ln]
                    )
                for hm in range(MG):
                    if (e, hm) in wv_tiles:
                        continue
                    load_wv(e, hm)
                wp8_sb = wpp.tile([128, KD, PCH, 2, 128], F8E4, tag="wp8")
                nc.sync.dma_start(wp8_sb[:], wp8_d[e])
                wp8lo_sb = wpp.tile([128, KD, PCH, 2, 128], F8E5, tag="wp8lo")
                nc.sync.dma_start(wp8lo_sb[:], wp8lo_d[e])
                wp_tiles[e] = (wp8_sb, wp8lo_sb)

            for e in range(E):
                eblocks = [b for b in blocks if b[0] == e]
                # gate/value matmuls + silu-mult into ht8/ht8lo. For e0 the
                # hm0/hm1 passes are interleaved per block so the PE's early
                # work lands on the tensors that arrive first (block 0/1 of
                # x plus two weight tiles), instead of needing all of e0's x
                # for hm0 up front.
                if e == 0:
                    hmblks = [(hm, b) for b in eblocks for hm in (0, 1)]
                    hmblks += [(hm, b) for hm in (2, 3) for b in eblocks]
                else:
                    hmblks = [(hm, b) for hm in range(MG) for b in eblocks]
                for hm, (_, c0, ln) in hmblks:
                    w8, w8lo = wv_tiles[(e, hm)]
                    if True:
                        psg = pg.tile([128, BLK], F32, tag="g")
                        psv = pv.tile([128, BLK], F32, tag="v")
                        for gv, ps, skips in (
                            (0, psg, SKIP_W_GATE),
                            (1, psv, SKIP_W_VAL),
                        ):
                            terms = []
                            for c in range(NCH):  # main
                                terms.append((w8, c, xte8))
                            for c in range(NCH):  # x-quantization corr
                                terms.append((w8, c, xte8lo))
                            for c in range(NCH):  # w-quantization corr
                                if (hm, c) not in skips:
                                    terms.append((w8lo, c, xte8))
                            for i, (wt, c, xt) in enumerate(terms):
                                nc.tensor.matmul(
                                    ps[:, :ln],
                                    lhsT=wt[:, gv, c, :, :],
                                    rhs=xt[:, c, :, c0 : c0 + ln],
                                    start=(i == 0),
                                    stop=(i == len(terms) - 1),
                                    perf_mode=PM.DoubleRow,
                                    skip_group_check=True,
                                )
                        sact = actp.tile([128, BLK], F32, tag="s")
                        nc.scalar.activation(sact[:, :ln], psg[:, :ln], AF.Silu)
                        h32 = actp.tile([128, BLK], F32, tag="h")
                        nc.vector.tensor_tensor(
                            out=h32[:, :ln],
                            in0=sact[:, :ln],
                            in1=psv[:, :ln],
                            op=OP.mult,
                        )
                        nc.scalar.activation(
                            ht8[:, hm, c0 : c0 + ln], h32[:, :ln], AF.Copy
                        )
                        nc.vector.tensor_tensor(
                            out=ht8lo[:, hm, c0 : c0 + ln],
                            in0=h32[:, :ln],
                            in1=ht8[:, hm, c0 : c0 + ln],
                            op=OP.subtract,
                        )
                # proj: per token block, all 8 d-tiles; copies on Pool
                wp8_sb, wp8lo_sb = wp_tiles[e]
                for (_, c0, ln) in eblocks:
                    ysb = outp.tile([128, KD, BLK], BF16, tag="y")
                    is_last = (e, c0, ln) == blocks[-1]
                    for d in range(KD):
                        psy = py.tile([128, BLK], F32, tag="py")
                        for c in range(PCH):
                            nc.tensor.matmul(
                                psy[:, :ln],
                                lhsT=wp8_sb[:, d, c, :, :],
                                rhs=ht8[:, 2 * c : 2 * c + 2, c0 : c0 + ln],
                                start=(c == 0),
                                stop=False,
                                perf_mode=PM.DoubleRow,
                                skip_group_check=True,
                            )
                        for c in range(PCH):
                            nc.tensor.matmul(
                                psy[:, :ln],
                                lhsT=wp8_sb[:, d, c, :, :],
                                rhs=ht8lo[:, 2 * c : 2 * c + 2, c0 : c0 + ln],
                                start=False,
                                stop=False,
                                perf_mode=PM.DoubleRow,
                                skip_group_check=True,
                            )
                        for c in range(PCH):
                            nc.tensor.matmul(
                                psy[:, :ln],
                                lhsT=wp8lo_sb[:, d, c, :, :],
                                rhs=ht8[:, 2 * c : 2 * c + 2, c0 : c0 + ln],
                                start=False,
                                stop=(c == PCH - 1),
                                perf_mode=PM.DoubleRow,
                                skip_group_check=True,
                            )
                        if d % 2 == 1 and not (is_last and d == KD - 1):
                            # copies alternate DVE/Act (GPSIMD cannot read
                            # PSUM); the final copy goes to DVE, which is
                            # idle by then, so the exit chain is short
                            nc.scalar.activation(
                                ysb[:, d, :ln], psy[:, :ln], AF.Copy
                            )
                        else:
                            nc.vector.tensor_copy(ysb[:, d, :ln], psy[:, :ln])
                        if is_last and d == KD - 2:
                            # drain d0..6 early so only d7's copy + a tiny
                            # DMA sit on the critical tail
                            nc.scalar.dma_start(
                                yt_d[:, : KD - 1, c0 : c0 + ln],
                                ysb[:, : KD - 1, :ln],
                            )
                    if is_last:
                        nc.sync.dma_start(
                            yt_d[:, KD - 1 :, c0 : c0 + ln],
                            ysb[:, KD - 1 :, :ln],
                        )
                    else:
                        nc.scalar.dma_start(
                            yt_d[:, :, c0 : c0 + ln], ysb[:, :, :ln]
                        )

    nc.compile()
    return nc


_NC = None
_NC_COUNTS = None


def _route(x, w_router):
    """Host router: f64 logits argmax (exactly matches the f32 reference
    argmax for any non-degenerate top-2 gap)."""
    x2 = np.asarray(x, dtype=np.float64).reshape(T, D)
    logits = x2 @ np.asarray(w_router, dtype=np.float64)
    eidx = np.argmax(logits, axis=1)
    counts = np.bincount(eidx, minlength=E)
    order = np.argsort(eidx, kind="stable")
    return eidx, counts, order


def _get_nc(counts=DEFAULT_COUNTS):
    global _NC, _NC_COUNTS
    counts = tuple(int(c) for c in counts)
    if _NC is None or _NC_COUNTS != counts:
        _NC = _build(counts)
        _NC_COUNTS = counts
    return _NC


def _split8(a):
    """hi = e4m3(a), lo = e5m2(a - hi); both at the caller's pre-scale."""
    hi = a.astype(F8NP)
    lo = (a - hi.astype(np.float32)).astype(F8E5NP)
    return hi, lo


def make_in_maps(x, w_v, w_proj, order):
    x2 = np.asarray(x, dtype=np.float32).reshape(T, D)
    wv = np.asarray(w_v, dtype=np.float32)
    wp = np.asarray(w_proj, dtype=np.float32)

    # compact transposed x: xtr8[p, c, j, t] = x[order[t], c*256+j*128+p] / 8
    xT = np.ascontiguousarray(x2[order].T)  # [D, T]
    x8, x8lo = _split8(xT / np.float32(S8))
    xtr8 = np.ascontiguousarray(
        x8.reshape(NCH, 2, 128, T).transpose(2, 0, 1, 3)
    )
    xtr8lo = np.ascontiguousarray(
        x8lo.reshape(NCH, 2, 128, T).transpose(2, 0, 1, 3)
    )

    in_maps = []
    for core in range(8):
        h0 = core * HS
        wv8_e = []
        wv8lo_e = []
        wp8_e = []
        wp8lo_e = []
        for e in range(E):
            for hm in range(MG):
                lo = h0 + hm * 128
                gcols = wv[e][:, lo : lo + 128]
                vcols = wv[e][:, H + lo : H + lo + 128]
                st = np.stack([gcols, vcols])  # [2, D, 128]
                q8, q8lo = _split8(st * np.float32(S8))
                # [gv, c, j, p, m] -> [p, gv, c, j, m]
                wv8_e.append(
                    q8.reshape(2, NCH, 2, 128, 128).transpose(3, 0, 1, 2, 4)
                )
                wv8lo_e.append(
                    q8lo.reshape(2, NCH, 2, 128, 128).transpose(3, 0, 1, 2, 4)
                )
            wp_my = wp[e][h0 : h0 + HS, :]  # [HS, D]
            p8, p8lo = _split8(wp_my * np.float32(S8))
            # [c, j, p, d, m] -> [p, d, c, j, m]
            wp8_e.append(
                p8.reshape(PCH, 2, 128, KD, 128).transpose(2, 3, 0, 1, 4)
            )
            wp8lo_e.append(
                p8lo.reshape(PCH, 2, 128, KD, 128).transpose(2, 3, 0, 1, 4)
            )
        im = {
            "xtr8": xtr8,
            "xtr8lo": xtr8lo,
            "wv8": np.ascontiguousarray(np.stack(wv8_e)),
            "wv8lo": np.ascontiguousarray(np.stack(wv8lo_e)),
            "wp8": np.ascontiguousarray(np.stack(wp8_e)),
            "wp8lo": np.ascontiguousarray(np.stack(wp8lo_e)),
        }
        in_maps.append(im)
    return in_maps


def combine(results, order):
    """Sum the 8 hidden-slice partial outputs (each 8*y), inverse-permute,
    and divide out the w_proj pre-scale."""
    ysum = np.zeros((128, KD, T), dtype=np.float32)
    for r in results:
        ysum += np.asarray(r["yt"]).astype(np.float32)
    ysum *= np.float32(1.0 / S8)
    yT = ysum.transpose(1, 0, 2).reshape(D, T)  # [D, T] compact order
    out = np.empty((T, D), dtype=np.float32)
    out[order] = yT.T
    return out.reshape(2, 2048, D)


def kernel(x, w_router, w_v, w_proj):
    eidx, counts, order = _route(x, w_router)
    nc = _get_nc(counts)
    in_maps = make_in_maps(x, w_v, w_proj, order)
    res = run_bass_kernel_spmd(nc, in_maps, core_ids=list(range(8)), trace=False)
    return combine(res.results, order)


if __name__ == "__main__":
    sys.path.insert(0, "/root/problem")
    import reference

    ins = {k: np.asarray(v) for k, v in reference.setup_inputs().items()}
    got = kernel(**ins)
    exp = np.asarray(reference.reference(**ins))
    err = np.abs(got - exp)
    denom = np.abs(exp).max()
    print("max abs err:", err.max(), "rel:", err.max() / denom)


# revision 23
# speedup vs baseline: 1.2755x; 1.0068x over previous
"""MoE top-1 routing kernel for Trainium2, 8 NeuronCores.

Problem: x [2, 2048, 1024] f32; router w [1024, 4]; per-expert SwiGLU MLP
  gv = x @ w_v[e] ([1024, 8192]); h = silu(gv[:, :4096]) * gv[:, 4096:];
  y = h @ w_proj[e] ([4096, 1024]); out[t] = y_{argmax(router)}[t].

Sharding: tokens are dispatched by expert_idx at the host sharding step
(router is 0.03% of total FLOPs; argmax computed in f64, which matches the
f32 reference argmax exactly). Tokens are permuted into expert-contiguous
order; every core receives ALL tokens plus a 1/8 slice of the hidden
dimension of EVERY expert's weights (hidden-slice model parallelism).
Per-core work is exactly total_tokens * (3*D*H/8) MACs regardless of
expert load imbalance, with zero capacity padding.

Numerics: every 256-deep contraction chunk of all three matmuls runs as
fp8 DoubleRow (0.5 PE cycles/row = 4x bf16) with a 3-term residual
decomposition that restores ~bf16 accuracy:
    main  e4m3(w*8)  x e4m3(x/8)     -- 4x-rate product of rounded values
    corrX e4m3(w*8)  x e5m2(x_lo/8)  -- cancels x's quantization error
    corrW e5m2(w_lo) x e4m3(x/8)     -- cancels w's quantization error
Residual error is (w_lo*x_lo + e5m2 rounding) ~ 2^-7 per element, measured
3.5e-3 max-rel end to end (threshold 2e-2). PE cost is 1.5 cycles per
256-chunk vs 2.0 for bf16: 72 cycles/token/core -> ~123us @2.4GHz.

The proj input h = silu(g)*v is produced on-device as the same e4m3+e5m2
pair: Act silu (psg->f32), DVE mult (x psv -> f32), Act copy->e4m3,
DVE subtract->e5m2. Proj PSUM->SBUF copies run on the otherwise idle Pool
engine. Output yt stores 8*y (w_proj is pre-scaled by 8); the host
combine divides by 8 after summing the 8 hidden-slice partials.
"""

import sys

sys.path.insert(0, "/opt/trn_rl_repo")

import ml_dtypes
import numpy as np

import concourse.bass as bass  # noqa: F401  (kept for parity with utils)
import concourse.mybir as mybir
import concourse.tile as tile
from concourse import bacc
from concourse.bass_utils import run_bass_kernel_spmd

F32 = mybir.dt.float32
BF16 = mybir.dt.bfloat16
F8E4 = mybir.dt.float8e4
F8E5 = mybir.dt.float8e5
PM = mybir.MatmulPerfMode
AF = mybir.ActivationFunctionType
OP = mybir.AluOpType
BF16NP = np.dtype(ml_dtypes.bfloat16)
F8NP = np.dtype(ml_dtypes.float8_e4m3)
F8E5NP = np.dtype(ml_dtypes.float8_e5m2)

T = 4096      # tokens
D = 1024      # model dim
E = 4         # experts
H = 4096      # MLP hidden (SwiGLU: w_v outputs 2*H)
HS = H // 8   # hidden slice per core
NCH = D // 256    # 4 fp8 256-chunks over model dim
PCH = HS // 256   # 2 fp8 256-chunks over the hidden slice (proj contraction)
MG = HS // 128    # 4 gate (and 4 value) 128-row tiles per slice
KD = D // 128     # 8 output d-tiles for proj
BLK = 512         # max token block (one PSUM bank of f32)
NWARM = 24        # PE warm-up dummy matmuls (cover the startup DMA window)
S8 = 8.0          # fp8 pre-scale: weights*S8, x/S8

# Per-token error units where the w-residual correction is skipped (the
# chunk runs 2-term: main + x-residual only). Each unit saves 0.5 PE
# cycles/token (~0.85us) and adds ~0.64e-2 (RSS) to the end-to-end rel
# error on top of the ~0.35e-2 3-term base. 6 units measured 1.71e-2 on
# device; 7 units -> ~1.83e-2 predicted (threshold 2e-2).
SKIP_W_GATE = {(0, 0), (1, 0), (2, 0), (3, 0)}   # (hm, chunk)
SKIP_W_VAL = {(0, 0), (1, 0), (2, 0)}

# Expert loads for the seed-0 reference data (default build).
DEFAULT_COUNTS = (1149, 902, 974, 1071)


def _blocks(counts):
    """Static block structure: (expert, col_start, col_len) over the compact
    token stream; ragged tails, no padding. Expert 0 leads with a small
    256-col block so the PE can start earlier (first DMA is smaller)."""
    out = []
    c0 = 0
    for e in range(E):
        n = int(counts[e])
        off = 0
        if e == 0 and n > 256:
            out.append((e, c0, 256))
            off = 256
        while off < n:
            ln = min(BLK, n - off)
            out.append((e, c0 + off, ln))
            off += ln
        c0 += n
    return out


def _build(counts):
    nc = bacc.Bacc("TRN2", target_bir_lowering=False, debug=False, num_devices=8)

    xtr8_d = nc.dram_tensor("xtr8", [128, NCH, 2, T], F8E4, kind="ExternalInput").ap()
    xtr8lo_d = nc.dram_tensor(
        "xtr8lo", [128, NCH, 2, T], F8E5, kind="ExternalInput"
    ).ap()
    wv8_d = nc.dram_tensor(
        "wv8", [E * MG, 128, 2, NCH, 2, 128], F8E4, kind="ExternalInput"
    ).ap()
    wv8lo_d = nc.dram_tensor(
        "wv8lo", [E * MG, 128, 2, NCH, 2, 128], F8E5, kind="ExternalInput"
    ).ap()
    wp8_d = nc.dram_tensor(
        "wp8", [E, 128, KD, PCH, 2, 128], F8E4, kind="ExternalInput"
    ).ap()
    wp8lo_d = nc.dram_tensor(
        "wp8lo", [E, 128, KD, PCH, 2, 128], F8E5, kind="ExternalInput"
    ).ap()
    yt_d = nc.dram_tensor("yt", [128, KD, T], BF16, kind="ExternalOutput").ap()

    blocks = _blocks(counts)

    with tile.TileContext(nc) as tc:
        with (
            tc.tile_pool(name="xte", bufs=1) as xp,
            tc.tile_pool(name="ht", bufs=1) as hp,
            tc.tile_pool(name="wv", bufs=4) as wvp,
            tc.tile_pool(name="wp", bufs=2) as wpp,
            tc.tile_pool(name="act", bufs=3) as actp,
            tc.tile_pool(name="out", bufs=3) as outp,
            tc.tile_pool(name="pg", bufs=3, space="PSUM") as pg,
            tc.tile_pool(name="pv", bufs=2, space="PSUM") as pv,
            tc.tile_pool(name="py", bufs=3, space="PSUM") as py,
        ):
            xte8 = xp.tile([128, NCH, 2, T], F8E4)
            xte8lo = xp.tile([128, NCH, 2, T], F8E5)
            ht8 = hp.tile([128, MG, T], F8E4)
            ht8lo = hp.tile([128, MG, T], F8E5)

            # PE warm-up: the Tensor engine runs at half clock until it has
            # been continuously busy for 3us. Dummy matmuls on a memset tile
            # keep it busy through the startup DMA window so all real
            # matmuls run at full p-state.
            warm = actp.tile([128, 128], BF16, tag="warm")
            nc.vector.memset(warm[:], 0.0)
            pwu = pg.tile([128, 128], F32, tag="g")
            for _ in range(NWARM):
                nc.tensor.matmul(
                    pwu[:], lhsT=warm[:], rhs=warm[:], start=True, stop=True
                )

            # Startup-critical DMAs on different queues (per-DMA sequencer
            # time is ~0.6us, so serializing them on one queue delays the
            # first matmul); everything else in consumption order.
            wv_tiles = {}
            wp_tiles = {}

            def load_wv(e, hm, eng8=None, englo=None):
                w8 = wvp.tile([128, 2, NCH, 2, 128], F8E4, tag="w8")
                (eng8 or nc.sync).dma_start(w8[:], wv8_d[e * MG + hm])
                w8lo = wvp.tile([128, 2, NCH, 2, 128], F8E5, tag="w8lo")
                (englo or nc.sync).dma_start(w8lo[:], wv8lo_d[e * MG + hm])
                wv_tiles[(e, hm)] = (w8, w8lo)

            first_blk = blocks[0]
            _, fc0, fln = first_blk
            e0b = [(c0, ln) for (ee, c0, ln) in blocks if ee == 0]
            # Startup: e0 runs hm0/hm1 interleaved per block, so the first
            # weights needed are (0,0) and (0,1). Their e4m3 parts lead the
            # SP queue. The Pool SWDGE queue (idle until e0's proj) leads
            # with block 0's x-residual -- the first correction input the
            # PE needs -- then the two weight residuals and block 1's
            # x-residual, all consumed a few DRs later. x block 0 rides the
            # Act queue behind the Silu table load.
            nc.gpsimd.dma_start(
                xte8lo[:, :, :, fc0 : fc0 + fln],
                xtr8lo_d[:, :, :, fc0 : fc0 + fln],
            )
            load_wv(0, 0, eng8=nc.sync, englo=nc.gpsimd)
            load_wv(0, 1, eng8=nc.sync, englo=nc.gpsimd)
            nc.scalar.dma_start(
                xte8[:, :, :, fc0 : fc0 + fln], xtr8_d[:, :, :, fc0 : fc0 + fln]
            )
            if len(e0b) > 1:
                b1c0, b1ln = e0b[1]
                nc.gpsimd.dma_start(
                    xte8lo[:, :, :, b1c0 : b1c0 + b1ln],
                    xtr8lo_d[:, :, :, b1c0 : b1c0 + b1ln],
                )
            # Remaining x-residuals on the Pool SWDGE queue: Pool's compute
            # (proj PSUM->SBUF copies) doesn't start until the first proj
            # block (~26us), so its queue is free during the load window.
            xlo_spans = []
            for e in range(E):
                ecols = [(c0, ln) for (ee, c0, ln) in blocks if ee == e]
                ec0 = ecols[0][0]
                ec1 = ecols[-1][0] + ecols[-1][1]
                if e == 0:
                    # blocks 0 and 1 already in flight on the Act queue
                    ec0 = min(ec1, e0b[1][0] + e0b[1][1] if len(e0b) > 1 else ec1)
                if ec1 > ec0:
                    xlo_spans.append((ec0, ec1))
            for (a, b) in xlo_spans:
                nc.gpsimd.dma_start(
                    xte8lo[:, :, :, a:b], xtr8lo_d[:, :, :, a:b]
                )

            for e in range(E):
                for (ee, c0, ln) in blocks:
                    if ee != e or (ee, c0, ln) == first_blk:
                        continue
                    nc.sync.dma_start(
                        xte8[:, :, :, c0 : c0 + ln], xtr8_d[:, :, :, c0 : c0 + ln]
                    )
                for hm in range(MG):
                    if (e, hm) in wv_tiles:
                        continue
                    load_wv(e, hm)
                wp8_sb = wpp.tile([128, KD, PCH, 2, 128], F8E4, tag="wp8")
                nc.sync.dma_start(wp8_sb[:], wp8_d[e])
                wp8lo_sb = wpp.tile([128, KD, PCH, 2, 128], F8E5, tag="wp8lo")
                nc.sync.dma_start(wp8lo_sb[:], wp8lo_d[e])
                wp_tiles[e] = (wp8_sb, wp8lo_sb)

            for e in range(E):
                eblocks = [b for b in blocks if b[0] == e]
                # gate/value matmuls + silu-mult into ht8/ht8lo. For e0 the
                # hm0/hm1 passes are interleaved per block so the PE's early
                # work lands on the tensors that arrive first (block 0/1 of
                # x plus two weight tiles), instead of needing all of e0's x
                # for hm0 up front.
                if e == 0:
                    hmblks = [(hm, b) for b in eblocks for hm in (0, 1)]
                    hmblks += [(hm, b) for hm in (2, 3) for b in eblocks]
                else:
                    hmblks = [(hm, b) for hm in range(MG) for b in eblocks]
                for hm, (_, c0, ln) in hmblks:
                    w8, w8lo = wv_tiles[(e, hm)]
                    if True:
                        psg = pg.tile([128, BLK], F32, tag="g")
                        psv = pv.tile([128, BLK], F32, tag="v")
                        for gv, ps, skips in (
                            (0, psg, SKIP_W_GATE),
                            (1, psv, SKIP_W_VAL),
                        ):
                            terms = []
                            for c in range(NCH):  # main
                                terms.append((w8, c, xte8))
                            for c in range(NCH):  # x-quantization corr
                                terms.append((w8, c, xte8lo))
                            for c in range(NCH):  # w-quantization corr
                                if (hm, c) not in skips:
                                    terms.append((w8lo, c, xte8))
                            for i, (wt, c, xt) in enumerate(terms):
                                nc.tensor.matmul(
                                    ps[:, :ln],
                                    lhsT=wt[:, gv, c, :, :],
                                    rhs=xt[:, c, :, c0 : c0 + ln],
                                    start=(i == 0),
                                    stop=(i == len(terms) - 1),
                                    perf_mode=PM.DoubleRow,
                                    skip_group_check=True,
                                )
                        sact = actp.tile([128, BLK], F32, tag="s")
                        nc.scalar.activation(sact[:, :ln], psg[:, :ln], AF.Silu)
                        h32 = actp.tile([128, BLK], F32, tag="h")
                        nc.vector.tensor_tensor(
                            out=h32[:, :ln],
                            in0=sact[:, :ln],
                            in1=psv[:, :ln],
                            op=OP.mult,
                        )
                        nc.scalar.activation(
                            ht8[:, hm, c0 : c0 + ln], h32[:, :ln], AF.Copy
                        )
                        nc.vector.tensor_tensor(
                            out=ht8lo[:, hm, c0 : c0 + ln],
                            in0=h32[:, :ln],
                            in1=ht8[:, hm, c0 : c0 + ln],
                            op=OP.subtract,
                        )
                # proj: per token block, all 8 d-tiles; copies on Pool
                wp8_sb, wp8lo_sb = wp_tiles[e]
                for (_, c0, ln) in eblocks:
                    ysb = outp.tile([128, KD, BLK], BF16, tag="y")
                    is_last = (e, c0, ln) == blocks[-1]
                    for d in range(KD):
                        psy = py.tile([128, BLK], F32, tag="py")
                        for c in range(PCH):
                            nc.tensor.matmul(
                                psy[:, :ln],
                                lhsT=wp8_sb[:, d, c, :, :],
                                rhs=ht8[:, 2 * c : 2 * c + 2, c0 : c0 + ln],
                                start=(c == 0),
                                stop=False,
                                perf_mode=PM.DoubleRow,
                                skip_group_check=True,
                            )
                        for c in range(PCH):
                            nc.tensor.matmul(
                                psy[:, :ln],
                                lhsT=wp8_sb[:, d, c, :, :],
                                rhs=ht8lo[:, 2 * c : 2 * c + 2, c0 : c0 + ln],
                                start=False,
                                stop=False,
                                perf_mode=PM.DoubleRow,
                                skip_group_check=True,
                            )
                        for c in range(PCH):
                            nc.tensor.matmul(
                                psy[:, :ln],
                                lhsT=wp8lo_sb[:, d, c, :, :],
                                rhs=ht8[:, 2 * c : 2 * c + 2, c0 : c0 + ln],
                                start=False,
                                stop=(c == PCH - 1),
                                perf_mode=PM.DoubleRow,
                                skip_group_check=True,
                            )
                        if d % 2 == 1 and not (is_last and d == KD - 1):
                            # copies alternate DVE/Act (GPSIMD cannot read
                            # PSUM); the final copy goes to DVE, which is
                            # idle by then, so the exit chain is short
                            nc.scalar.activation(
                                ysb[:, d, :ln], psy[:, :ln], AF.Copy
                            )
                        else:
                            nc.vector.tensor_copy(ysb[:, d, :ln], psy[:, :ln])
                        if is_last and d == KD - 2:
                            # drain d0..6 early so only d7's copy + a tiny
                            # DMA sit on the critical tail
                            nc.scalar.dma_start(
                                yt_d[:, : KD - 1, c0 : c0 + ln],
                                ysb[:, : KD - 1, :ln],
                            )
                    if is_last:
                        nc.sync.dma_start(
                            yt_d[:, KD - 1 :, c0 : c0 + ln],
                            ysb[:, KD - 1 :, :ln],
                        )
                    else:
                        nc.scalar.dma_start(
                            yt_d[:, :, c0 : c0 + ln], ysb[:, :, :ln]
                        )

    nc.compile()
    return nc


_NC = None
_NC_COUNTS = None


def _route(x, w_router):
    """Host router: f64 logits argmax (exactly matches the f32 reference
    argmax for any non-degenerate top-2 gap)."""
    x2 = np.asarray(x, dtype=np.float64).reshape(T, D)
    logits = x2 @ np.asarray(w_router, dtype=np.float64)
    eidx = np.argmax(logits, axis=1)
    counts = np.bincount(eidx, minlength=E)
    order = np.argsort(eidx, kind="stable")
    return eidx, counts, order


def _get_nc(counts=DEFAULT_COUNTS):
    global _NC, _NC_COUNTS
    counts = tuple(int(c) for c in counts)
    if _NC is None or _NC_COUNTS != counts:
        _NC = _build(counts)
        _NC_COUNTS = counts
    return _NC


def _split8(a):
    """hi = e4m3(a), lo = e5m2(a - hi); both at the caller's pre-scale."""
    hi = a.astype(F8NP)
    lo = (a - hi.astype(np.float32)).astype(F8E5NP)
    return hi, lo


def make_in_maps(x, w_v, w_proj, order):
    x2 = np.asarray(x, dtype=np.float32).reshape(T, D)
    wv = np.asarray(w_v, dtype=np.float32)
    wp = np.asarray(w_proj, dtype=np.float32)

    # compact transposed x: xtr8[p, c, j, t] = x[order[t], c*256+j*128+p] / 8
    xT = np.ascontiguousarray(x2[order].T)  # [D, T]
    x8, x8lo = _split8(xT / np.float32(S8))
    xtr8 = np.ascontiguousarray(
        x8.reshape(NCH, 2, 128, T).transpose(2, 0, 1, 3)
    )
    xtr8lo = np.ascontiguousarray(
        x8lo.reshape(NCH, 2, 128, T).transpose(2, 0, 1, 3)
    )

    in_maps = []
    for core in range(8):
        h0 = core * HS
        wv8_e = []
        wv8lo_e = []
        wp8_e = []
        wp8lo_e = []
        for e in range(E):
            for hm in range(MG):
                lo = h0 + hm * 128
                gcols = wv[e][:, lo : lo + 128]
                vcols = wv[e][:, H + lo : H + lo + 128]
                st = np.stack([gcols, vcols])  # [2, D, 128]
                q8, q8lo = _split8(st * np.float32(S8))
                # [gv, c, j, p, m] -> [p, gv, c, j, m]
                wv8_e.append(
                    q8.reshape(2, NCH, 2, 128, 128).transpose(3, 0, 1, 2, 4)
                )
                wv8lo_e.append(
                    q8lo.reshape(2, NCH, 2, 128, 128).transpose(3, 0, 1, 2, 4)
                )
            wp_my = wp[e][h0 : h0 + HS, :]  # [HS, D]
            p8, p8lo = _split8(wp_my * np.float32(S8))
            # [c, j, p, d, m] -> [p, d, c, j, m]
            wp8_e.append(
                p8.reshape(PCH, 2, 128, KD, 128).transpose(2, 3, 0, 1, 4)
            )
            wp8lo_e.append(
                p8lo.reshape(PCH, 2, 128, KD, 128).transpose(2, 3, 0, 1, 4)
            )
        im = {
            "xtr8": xtr8,
            "xtr8lo": xtr8lo,
            "wv8": np.ascontiguousarray(np.stack(wv8_e)),
            "wv8lo": np.ascontiguousarray(np.stack(wv8lo_e)),
            "wp8": np.ascontiguousarray(np.stack(wp8_e)),
            "wp8lo": np.ascontiguousarray(np.stack(wp8lo_e)),
        }
        in_maps.append(im)
    return in_maps


def combine(results, order):
    """Sum the 8 hidden-slice partial outputs (each 8*y), inverse-permute,
    and divide out the w_proj pre-scale."""
    ysum = np.zeros((128, KD, T), dtype=np.float32)
    for r in results:
        ysum += np.asarray(r["yt"]).astype(np.float32)
    ysum *= np.float32(1.0 / S8)
    yT = ysum.transpose(1, 0, 2).reshape(D, T)  # [D, T] compact order
    out = np.empty((T, D), dtype=np.float32)
    out[order] = yT.T
    return out.reshape(2, 2048, D)


def kernel(x, w_router, w_v, w_proj):
    eidx, counts, order = _route(x, w_router)
    nc = _get_nc(counts)
    in_maps = make_in_maps(x, w_v, w_proj, order)
    res = run_bass_kernel_spmd(nc, in_maps, core_ids=list(range(8)), trace=False)
    return combine(res.results, order)


if __name__ == "__main__":
    sys.path.insert(0, "/root/problem")
    import reference

    ins = {k: np.asarray(v) for k, v in reference.setup_inputs().items()}
    got = kernel(**ins)
    exp = np.asarray(reference.reference(**ins))
    err = np.abs(got - exp)
    denom = np.abs(exp).max()
    print("max abs err:", err.max(), "rel:", err.max() / denom)
